# revision 8
# baseline (speedup 1.0000x reference)
"""Trainium2 Bass kernel for nn_Attention_1503238553757 (LSA attention).

Reference computation (per batch element):
    qkv = x @ w_qkv; q,k,v heads of dim 64
    dots = (q @ k^T) * scale[h]; diagonal masked to -inf
    attn = softmax(dots); out = attn @ v
    y = concat_heads(out) @ w_out + b_out

Sharding: data-parallel over batch (16 batches -> 2 per core x 8 cores).

Per-core plan (all matmuls fp32r = full-rate tf32-like):
  - x [1024, 512] loaded token-major, transposed on PE -> xT [512, 1024]
  - qT,kT channel-major via lhsT=w_qkv, rhs=xT    (scoresT needs ch-major)
  - v token-major via lhsT=xT, rhs=w_qkv[:, v]    (attn@V lhsT needs tok-major)
  - scoresT[j, i] = kT_h-slice @ qT_h  (keys on partitions); heads processed
    in pairs occupying PE row-groups 0-63 / 64-127 concurrently
  - expT = exp(scale_h * scoresT) via ACT (PSUM->SBUF), diag zeroed via
    affine_select (LSA self-token mask)
  - attn@V: lhsT = [v_h | ones] (M=65) accumulated over j-tiles ->
    outT[0:64] = unnormalized out^T, outT[64] = softmax denominators
  - normalize: fast reciprocal + DRAM-bounce partition-broadcast + DVE
    multiply, written as oT (inner-channel-major) = lhsT for out projection
  - y = oT.T @ w_out + b_out, token-major, DMA'd out

Emission is software-pipelined to keep the PE dense (HAM warm): attention
for batch b starts after a minimal projection prefix; batch b+1's x-load
and transposes interleave into batch b's attention pairs; batch b's out
projection interleaves into batch b+1's attention.

PSUM: psB ([128,1024] = 2 banks) x2 + psO ([65,1024] = 2 banks) x2 = 8 banks.
"""

import os
import sys

for _p in ("/opt/trn_rl_repo", "/root/.axon_site/_ro/trn_rl_repo"):
    if os.path.isdir(_p) and _p not in sys.path:
        sys.path.insert(0, _p)

import numpy as np

import concourse.bass as bass
import concourse.bacc as bacc
import concourse.tile as tile
import concourse.mybir as mybir
from concourse.bass_utils import run_bass_kernel_spmd

# Problem constants (hardcoded per harness contract)
B, N, D = 16, 1024, 512
HEADS, DH = 8, 64
N_CORES = 8
BPC = B // N_CORES  # batches per core = 2

dt = mybir.dt
F32 = dt.float32
F32R = dt.float32r
EXP = mybir.ActivationFunctionType.Exp

NT = N // 128   # token tiles = 8
KD = D // 128   # d/inner k-tiles = 4


def build_program():
    nc = bacc.Bacc("TRN2", target_bir_lowering=False, debug=False,
                   num_devices=N_CORES)

    x = nc.dram_tensor("x", [BPC, N, D], F32, kind="ExternalInput").ap()
    w_qkv = nc.dram_tensor("w_qkv", [D, 3 * D], F32, kind="ExternalInput").ap()
    w_out = nc.dram_tensor("w_out", [D, D], F32, kind="ExternalInput").ap()
    b_out = nc.dram_tensor("b_out", [D], F32, kind="ExternalInput").ap()
    scale = nc.dram_tensor("scale", [HEADS], F32, kind="ExternalInput").ap()
    y = nc.dram_tensor("y", [BPC, N, D], F32, kind="ExternalOutput").ap()

    ident_dram = nc.inline_tensor(np.eye(128, dtype=np.float32), name="ident")
    ones_dram = nc.inline_tensor(np.ones((128, 64), dtype=np.float32), name="ones128")

    import contextlib
    with tile.TileContext(nc) as tc, contextlib.ExitStack() as ctx:
        consts = ctx.enter_context(tc.tile_pool(name="consts", bufs=1))
        p_x = ctx.enter_context(tc.tile_pool(name="p_x", bufs=1))
        p_exp = ctx.enter_context(tc.tile_pool(name="p_exp", bufs=4))
        p_mid = ctx.enter_context(tc.tile_pool(name="p_mid", bufs=3))
        p_qk = ctx.enter_context(tc.tile_pool(name="p_qk", bufs=1))
        p_v = ctx.enter_context(tc.tile_pool(name="p_v", bufs=1))
        p_y = ctx.enter_context(tc.tile_pool(name="p_y", bufs=3))
        p_rb = ctx.enter_context(tc.tile_pool(name="p_rb", bufs=2))
        p_otmp = ctx.enter_context(tc.tile_pool(name="p_otmp", bufs=3))
        p_small = ctx.enter_context(tc.tile_pool(name="p_small", bufs=2))
        psB = ctx.enter_context(tc.tile_pool(name="psB", bufs=2, space="PSUM"))
        psO = ctx.enter_context(tc.tile_pool(name="psO", bufs=2, space="PSUM"))
        p_dram = ctx.enter_context(tc.tile_pool(name="p_dram", bufs=2, space="DRAM"))

        # ---- constants (weights on the scalar HWDGE queue so the x load
        # on the sync queue starts immediately) ----
        wqkv_sb = consts.tile([128, KD, 3 * D], F32R)
        nc.scalar.dma_start(
            out=wqkv_sb,
            in_=w_qkv.rearrange("(k p) c -> p k c", p=128).bitcast(F32R),
        )
        wout_sb = consts.tile([128, KD, D], F32R)
        nc.scalar.dma_start(
            out=wout_sb,
            in_=w_out.rearrange("(k p) c -> p k c", p=128).bitcast(F32R),
        )
        ident_sb = consts.tile([128, 128], F32R)
        nc.sync.dma_start(out=ident_sb, in_=ident_dram.ap().bitcast(F32R))
        bout_bc = consts.tile([128, D], F32)
        nc.gpsimd.dma_start(
            out=bout_bc,
            in_=bass.AP(tensor=b_out.tensor, offset=0, ap=[[0, 128], [1, D]]),
        )
        scale_sb = consts.tile([128, HEADS], F32)
        nc.gpsimd.dma_start(
            out=scale_sb,
            in_=bass.AP(tensor=scale.tensor, offset=0, ap=[[0, 128], [1, HEADS]]),
        )

        # per-batch state kept across the pipelined emission
        xT = [None] * BPC
        qkT = [None] * BPC
        vsb = [None] * BPC
        osb = [None] * BPC

        def emit_load_x(b):
            x_sb = p_x.tile([128, NT, D], F32R, tag="x")
            nc.sync.dma_start(
                out=x_sb,
                in_=x[b].rearrange("(r p) d -> p r d", p=128).bitcast(F32R),
            )
            return x_sb

        def emit_transposes(b, x_sb, kds):
            for kd in kds:
                ps_t = psB.tile([128, N], F32R, tag="psB")
                for r in range(NT):
                    nc.tensor.transpose(
                        ps_t[:, 128 * r:128 * r + 128],
                        x_sb[:, r, 128 * kd:128 * kd + 128],
                        ident_sb,
                    )
                nc.vector.tensor_copy(xT[b][:, kd, :], ps_t)

        def emit_qk_ct(b, ct):
            ps_qk = psB.tile([128, N], F32, tag="psB")
            for kt in range(KD):
                for nh in range(2):
                    nc.tensor.matmul(
                        ps_qk[:, 512 * nh:512 * nh + 512],
                        wqkv_sb[:, kt, 128 * ct:128 * ct + 128],
                        xT[b][:, kt, 512 * nh:512 * nh + 512],
                        start=(kt == 0), stop=(kt == KD - 1),
                    )
            nc.vector.tensor_copy(qkT[b][:, ct, :], ps_qk)

        def emit_v_group(b, g):
            ps_v = psB.tile([128, N], F32, tag="psB")
            for rr in range(2):
                r = 2 * g + rr
                for kt in range(KD):
                    nc.tensor.matmul(
                        ps_v[:, 512 * rr:512 * rr + 512],
                        xT[b][:, kt, 128 * r:128 * r + 128],
                        wqkv_sb[:, kt, 2 * D:3 * D],
                        start=(kt == 0), stop=(kt == KD - 1),
                    )
            nc.vector.tensor_copy(
                vsb[b][:, 2 * g:2 * g + 2, :, 0:DH],
                ps_v.rearrange("p (r2 h e) -> p r2 h e", r2=2, h=HEADS),
            )

        def emit_ones(b):
            nc.sync.dma_start(
                out=vsb[b][:, :, :, DH:DH + 1].bitcast(F32),
                in_=ones_dram.ap()[:, 0:NT * HEADS].rearrange(
                    "p (r h) -> p r h", r=NT
                ).unsqueeze(3),
            )

        def emit_head_pair(b, g, filler=None):
            """Attention for heads (2g, 2g+1) of batch b; the two heads
            occupy PE row groups 0-63 / 64-127 concurrently.
            filler() is called once mid-pair to interleave other work."""
            heads = (2 * g, 2 * g + 1)
            ps_os = {h: psO.tile([DH + 1, N], F32, tag="psO", name=f"ps_o_{b}_{h}") for h in heads}
            for jt in range(NT):
                tiles = {}
                for h in heads:
                    q_off = (h % 2) * 64
                    ps_s = psB.tile([128, N], F32, tag="psB", name=f"ps_s_{b}_{h}_{jt}")
                    tiles[h] = ps_s
                    for ih in range(2):
                        nc.tensor.matmul(
                            ps_s[:, 512 * ih:512 * ih + 512],
                            qkT[b][q_off:q_off + 64, 4 + g,
                                   128 * jt:128 * jt + 128],
                            qkT[b][q_off:q_off + 64, g,
                                   512 * ih:512 * ih + 512],
                            start=True, stop=True,
                        )
                for h in heads:
                    expT = p_exp.tile([128, N], F32R, tag="exp")
                    nc.scalar.activation(
                        expT, tiles[h], EXP, scale=scale_sb[:, h:h + 1]
                    )
                    nc.gpsimd.affine_select(
                        out=expT[:, 128 * jt:128 * jt + 128],
                        in_=expT[:, 128 * jt:128 * jt + 128],
                        compare_op=mybir.AluOpType.not_equal,
                        fill=0.0, base=0, channel_multiplier=1,
                        pattern=[[-1, 128]],
                    )
                    for ih in range(2):
                        nc.tensor.matmul(
                            ps_os[h][:, 512 * ih:512 * ih + 512],
                            vsb[b][:, jt, h, :],
                            expT[:, 512 * ih:512 * ih + 512],
                            start=(jt == 0), stop=(jt == NT - 1),
                        )
                if jt == 3 and filler is not None:
                    filler()
            for h in heads:
                q_off = (h % 2) * 64
                # free the PSUM slot fast: single copy of out^T + sums row
                o_tmp = p_otmp.tile([DH + 1, N], F32, tag="otmp",
                                    name=f"o_tmp_{b}_{h}")
                nc.vector.tensor_copy(o_tmp, ps_os[h])
                sums_sb = p_small.tile([1, N], F32, tag="sums")
                nc.vector.tensor_copy(sums_sb, o_tmp[DH:DH + 1, :])
                recip = p_small.tile([1, N], F32, tag="recip")
                nc.vector.reciprocal_approx_fast(recip, sums_sb)
                scr = p_dram.tile([1, N], F32, tag="scr")
                nc.scalar.dma_start(out=scr, in_=recip)
                rb = p_rb.tile([64, N], F32, tag="rb")
                nc.gpsimd.dma_start(
                    out=rb,
                    in_=bass.AP(tensor=scr.tensor, offset=scr.offset,
                                ap=[[0, 64], [1, N]]),
                )
                nc.vector.tensor_mul(
                    osb[b][q_off:q_off + 64, g, :], o_tmp[0:DH, :], rb
                )

        def emit_yproj_group(b, g):
            ps_y = psB.tile([128, N], F32, tag="psB")
            for rr in range(2):
                r = 2 * g + rr
                for kt in range(KD):
                    nc.tensor.matmul(
                        ps_y[:, 512 * rr:512 * rr + 512],
                        osb[b][:, kt, 128 * r:128 * r + 128],
                        wout_sb[:, kt, :],
                        start=(kt == 0), stop=(kt == KD - 1),
                    )
            for rr in range(2):
                r = 2 * g + rr
                y_sb = p_y.tile([128, D], F32, tag="y")
                nc.vector.tensor_add(
                    y_sb, ps_y[:, 512 * rr:512 * rr + 512], bout_bc
                )
                nc.scalar.dma_start(
                    out=y[b, 128 * r:128 * r + 128, :], in_=y_sb
                )

        # ================= pipelined emission =================
        # batch 0 prologue: load + transpose + minimal projection prefix
        x0 = emit_load_x(0)
        xT[0] = p_mid.tile([128, KD, N], F32R, tag="mid", name="xT0")
        qkT[0] = p_qk.tile([128, 8, N], F32R, tag="qk", name="qkT0")
        vsb[0] = p_v.tile([128, NT, HEADS, DH + 1], F32R, tag="v", name="v0")
        osb[0] = p_mid.tile([128, KD, N], F32R, tag="mid", name="o0")
        emit_transposes(0, x0, range(KD))
        emit_ones(0)
        emit_qk_ct(0, 0)       # q heads 0,1
        emit_qk_ct(0, 4)       # k heads 0,1
        for g in range(4):
            emit_v_group(0, g)

        # batch 1 x-load can start as soon as x0's slot frees
        x1 = emit_load_x(1)
        xT[1] = p_mid.tile([128, KD, N], F32R, tag="mid", name="xT1")

        # C(0) pairs with remaining B(0) chunks + A(1) transposes interleaved
        fillers0 = [
            lambda: (emit_qk_ct(0, 1), emit_qk_ct(0, 5)),
            lambda: (emit_qk_ct(0, 2), emit_qk_ct(0, 6),
                     emit_transposes(1, x1, [0, 1])),
            lambda: (emit_qk_ct(0, 3), emit_qk_ct(0, 7),
                     emit_transposes(1, x1, [2, 3])),
            None,
        ]
        for g in range(4):
            emit_head_pair(0, g, filler=fillers0[g])

        # B(1) projections (PE-dense stretch between the two attention phases)
        qkT[1] = p_qk.tile([128, 8, N], F32R, tag="qk", name="qkT1")
        vsb[1] = p_v.tile([128, NT, HEADS, DH + 1], F32R, tag="v", name="v1")
        osb[1] = p_mid.tile([128, KD, N], F32R, tag="mid", name="o1")
        emit_ones(1)
        emit_qk_ct(1, 0)
        emit_qk_ct(1, 4)
        for g in range(4):
            emit_v_group(1, g)

        # C(1) pairs with D(0) + remaining B(1) chunks interleaved
        fillers1 = [
            lambda: (emit_qk_ct(1, 1), emit_qk_ct(1, 5),
                     emit_yproj_group(0, 0)),
            lambda: (emit_qk_ct(1, 2), emit_qk_ct(1, 6),
                     emit_yproj_group(0, 1)),
            lambda: (emit_qk_ct(1, 3), emit_qk_ct(1, 7),
                     emit_yproj_group(0, 2)),
            lambda: emit_yproj_group(0, 3),
        ]
        for g in range(4):
            emit_head_pair(1, g, filler=fillers1[g])

        # D(1) tail
        for g in range(4):
            emit_yproj_group(1, g)

    nc.compile()
    return nc


_NC = None


def _get_program():
    global _NC
    if _NC is None:
        _NC = build_program()
    return _NC


def make_in_maps(x, w_qkv, w_out, b_out, scale):
    x = np.ascontiguousarray(np.asarray(x, dtype=np.float32))
    w_qkv = np.ascontiguousarray(np.asarray(w_qkv, dtype=np.float32))
    w_out = np.ascontiguousarray(np.asarray(w_out, dtype=np.float32))
    b_out = np.ascontiguousarray(np.asarray(b_out, dtype=np.float32))
    scale = np.ascontiguousarray(np.asarray(scale, dtype=np.float32))
    return [
        {
            "x": x[c * BPC:(c + 1) * BPC],
            "w_qkv": w_qkv,
            "w_out": w_out,
            "b_out": b_out,
            "scale": scale,
        }
        for c in range(N_CORES)
    ]


def kernel(x, w_qkv, w_out, b_out, scale):
    nc = _get_program()
    in_maps = make_in_maps(x, w_qkv, w_out, b_out, scale)
    res = run_bass_kernel_spmd(nc, in_maps, core_ids=list(range(N_CORES)))
    return np.concatenate([res.results[c]["y"] for c in range(N_CORES)], axis=0)


if __name__ == "__main__":
    rng = np.random.default_rng(0)
    inputs = {
        "x": rng.standard_normal((B, N, D), dtype=np.float32),
        "w_qkv": rng.standard_normal((D, 3 * D), dtype=np.float32) * 0.03,
        "w_out": rng.standard_normal((D, D), dtype=np.float32) * 0.04,
        "b_out": np.zeros(D, dtype=np.float32),
        "scale": np.full(HEADS, DH ** -0.5, dtype=np.float32),
    }
    out = kernel(**inputs)
    print("kernel output", out.shape, out.dtype)


# revision 9
# speedup vs baseline: 1.1586x; 1.1586x over previous
"""Trainium2 Bass kernel for nn_Attention_1503238553757 (LSA attention).

Reference computation (per batch element):
    qkv = x @ w_qkv; q,k,v heads of dim 64
    dots = (q @ k^T) * scale[h]; diagonal masked to -inf
    attn = softmax(dots); out = attn @ v
    y = concat_heads(out) @ w_out + b_out

Sharding: data-parallel over batch (16 batches -> 2 per core x 8 cores).

Per-core plan (all matmuls fp32r = full-rate tf32-like):
  - x [1024, 512] loaded token-major, transposed on PE -> xT [512, 1024]
  - qT,kT channel-major via lhsT=w_qkv, rhs=xT    (scoresT needs ch-major)
  - v token-major via lhsT=xT, rhs=w_qkv[:, v]    (attn@V lhsT needs tok-major)
  - scoresT[j, i] = kT_h-slice @ qT_h  (keys on partitions); heads processed
    in pairs occupying PE row-groups 0-63 / 64-127 concurrently
  - expT = exp(scale_h * scoresT) via ACT (PSUM->SBUF), diag zeroed via
    affine_select (LSA self-token mask)
  - attn@V: lhsT = [v_h | ones] (M=65) accumulated over j-tiles ->
    outT[0:64] = unnormalized out^T, outT[64] = softmax denominators
  - normalize: fast reciprocal + DRAM-bounce partition-broadcast + DVE
    multiply, written as oT (inner-channel-major) = lhsT for out projection
  - y = oT.T @ w_out + b_out, token-major, DMA'd out

Emission is software-pipelined to keep the PE dense (HAM warm): attention
for batch b starts after a minimal projection prefix; batch b+1's x-load
and transposes interleave into batch b's attention pairs; batch b's out
projection interleaves into batch b+1's attention.

PSUM: psB ([128,1024] = 2 banks) x2 + psO ([65,1024] = 2 banks) x2 = 8 banks.
"""

import os
import sys

for _p in ("/opt/trn_rl_repo", "/root/.axon_site/_ro/trn_rl_repo"):
    if os.path.isdir(_p) and _p not in sys.path:
        sys.path.insert(0, _p)

import numpy as np

import concourse.bass as bass
import concourse.bacc as bacc
import concourse.tile as tile
import concourse.mybir as mybir
from concourse.bass_utils import run_bass_kernel_spmd

# Problem constants (hardcoded per harness contract)
B, N, D = 16, 1024, 512
HEADS, DH = 8, 64
N_CORES = 8
BPC = B // N_CORES  # batches per core = 2

dt = mybir.dt
F32 = dt.float32
F32R = dt.float32r
EXP = mybir.ActivationFunctionType.Exp

NT = N // 128   # token tiles = 8
KD = D // 128   # d/inner k-tiles = 4


def build_program():
    nc = bacc.Bacc("TRN2", target_bir_lowering=False, debug=False,
                   num_devices=N_CORES)

    x = nc.dram_tensor("x", [BPC, N, D], F32, kind="ExternalInput").ap()
    w_qkv = nc.dram_tensor("w_qkv", [D, 3 * D], F32, kind="ExternalInput").ap()
    w_out = nc.dram_tensor("w_out", [D, D], F32, kind="ExternalInput").ap()
    b_out = nc.dram_tensor("b_out", [D], F32, kind="ExternalInput").ap()
    scale = nc.dram_tensor("scale", [HEADS], F32, kind="ExternalInput").ap()
    y = nc.dram_tensor("y", [BPC, N, D], F32, kind="ExternalOutput").ap()

    ident_dram = nc.inline_tensor(np.eye(128, dtype=np.float32), name="ident")
    ones_dram = nc.inline_tensor(np.ones((128, 64), dtype=np.float32), name="ones128")

    import contextlib
    with tile.TileContext(nc) as tc, contextlib.ExitStack() as ctx:
        consts = ctx.enter_context(tc.tile_pool(name="consts", bufs=1))
        p_x = ctx.enter_context(tc.tile_pool(name="p_x", bufs=1))
        p_exp = ctx.enter_context(tc.tile_pool(name="p_exp", bufs=4))
        p_mid = ctx.enter_context(tc.tile_pool(name="p_mid", bufs=3))
        p_qk = ctx.enter_context(tc.tile_pool(name="p_qk", bufs=1))
        p_v = ctx.enter_context(tc.tile_pool(name="p_v", bufs=1))
        p_y = ctx.enter_context(tc.tile_pool(name="p_y", bufs=3))
        p_rb = ctx.enter_context(tc.tile_pool(name="p_rb", bufs=2))
        p_otmp = ctx.enter_context(tc.tile_pool(name="p_otmp", bufs=3))
        p_small = ctx.enter_context(tc.tile_pool(name="p_small", bufs=2))
        psB = ctx.enter_context(tc.tile_pool(name="psB", bufs=2, space="PSUM"))
        psO = ctx.enter_context(tc.tile_pool(name="psO", bufs=2, space="PSUM"))
        p_dram = ctx.enter_context(tc.tile_pool(name="p_dram", bufs=2, space="DRAM"))

        # ---- constants (weights on the scalar HWDGE queue so the x load
        # on the sync queue starts immediately) ----
        wqkv_sb = consts.tile([128, KD, 3 * D], F32R)
        nc.sync.dma_start(
            out=wqkv_sb,
            in_=w_qkv.rearrange("(k p) c -> p k c", p=128).bitcast(F32R),
        )
        wout_sb = consts.tile([128, KD, D], F32R)
        nc.sync.dma_start(
            out=wout_sb,
            in_=w_out.rearrange("(k p) c -> p k c", p=128).bitcast(F32R),
        )
        ident_sb = consts.tile([128, 128], F32R)
        nc.sync.dma_start(out=ident_sb, in_=ident_dram.ap().bitcast(F32R))
        bout_bc = consts.tile([128, D], F32)
        nc.gpsimd.dma_start(
            out=bout_bc,
            in_=bass.AP(tensor=b_out.tensor, offset=0, ap=[[0, 128], [1, D]]),
        )
        scale_sb = consts.tile([128, HEADS], F32)
        nc.gpsimd.dma_start(
            out=scale_sb,
            in_=bass.AP(tensor=scale.tensor, offset=0, ap=[[0, 128], [1, HEADS]]),
        )

        # per-batch state kept across the pipelined emission
        xT = [None] * BPC
        qkT = [None] * BPC
        vsb = [None] * BPC
        osb = [None] * BPC

        def emit_load_x(b):
            x_sb = p_x.tile([128, NT, D], F32R, tag="x")
            nc.sync.dma_start(
                out=x_sb,
                in_=x[b].rearrange("(r p) d -> p r d", p=128).bitcast(F32R),
            )
            return x_sb

        def emit_transposes(b, x_sb, kds):
            for kd in kds:
                ps_t = psB.tile([128, N], F32R, tag="psB")
                for r in range(NT):
                    nc.tensor.transpose(
                        ps_t[:, 128 * r:128 * r + 128],
                        x_sb[:, r, 128 * kd:128 * kd + 128],
                        ident_sb,
                    )
                nc.vector.tensor_copy(xT[b][:, kd, :], ps_t)

        def emit_qk_ct(b, ct):
            ps_qk = psB.tile([128, N], F32, tag="psB")
            for kt in range(KD):
                for nh in range(2):
                    nc.tensor.matmul(
                        ps_qk[:, 512 * nh:512 * nh + 512],
                        wqkv_sb[:, kt, 128 * ct:128 * ct + 128],
                        xT[b][:, kt, 512 * nh:512 * nh + 512],
                        start=(kt == 0), stop=(kt == KD - 1),
                    )
            nc.vector.tensor_copy(qkT[b][:, ct, :], ps_qk)

        def emit_v_group(b, g):
            ps_v = psB.tile([128, N], F32, tag="psB")
            for rr in range(2):
                r = 2 * g + rr
                for kt in range(KD):
                    nc.tensor.matmul(
                        ps_v[:, 512 * rr:512 * rr + 512],
                        xT[b][:, kt, 128 * r:128 * r + 128],
                        wqkv_sb[:, kt, 2 * D:3 * D],
                        start=(kt == 0), stop=(kt == KD - 1),
                    )
            nc.vector.tensor_copy(
                vsb[b][:, 2 * g:2 * g + 2, :, 0:DH],
                ps_v.rearrange("p (r2 h e) -> p r2 h e", r2=2, h=HEADS),
            )

        def emit_ones(b):
            nc.sync.dma_start(
                out=vsb[b][:, :, :, DH:DH + 1].bitcast(F32),
                in_=ones_dram.ap()[:, 0:NT * HEADS].rearrange(
                    "p (r h) -> p r h", r=NT
                ).unsqueeze(3),
            )

        def emit_head_pair(b, g, filler=None):
            """Attention for heads (2g, 2g+1) of batch b; the two heads
            occupy PE row groups 0-63 / 64-127 concurrently.
            filler() is called once mid-pair to interleave other work."""
            heads = (2 * g, 2 * g + 1)
            ps_os = {h: psO.tile([DH + 1, N], F32, tag="psO", name=f"ps_o_{b}_{h}") for h in heads}
            for jt in range(NT):
                tiles = {}
                for h in heads:
                    q_off = (h % 2) * 64
                    ps_s = psB.tile([128, N], F32, tag="psB", name=f"ps_s_{b}_{h}_{jt}")
                    tiles[h] = ps_s
                    for ih in range(2):
                        nc.tensor.matmul(
                            ps_s[:, 512 * ih:512 * ih + 512],
                            qkT[b][q_off:q_off + 64, 4 + g,
                                   128 * jt:128 * jt + 128],
                            qkT[b][q_off:q_off + 64, g,
                                   512 * ih:512 * ih + 512],
                            start=True, stop=True,
                        )
                for h in heads:
                    expT = p_exp.tile([128, N], F32R, tag="exp")
                    nc.scalar.activation(
                        expT, tiles[h], EXP, scale=scale_sb[:, h:h + 1]
                    )
                    nc.gpsimd.affine_select(
                        out=expT[:, 128 * jt:128 * jt + 128],
                        in_=expT[:, 128 * jt:128 * jt + 128],
                        compare_op=mybir.AluOpType.not_equal,
                        fill=0.0, base=0, channel_multiplier=1,
                        pattern=[[-1, 128]],
                    )
                    for ih in range(2):
                        nc.tensor.matmul(
                            ps_os[h][:, 512 * ih:512 * ih + 512],
                            vsb[b][:, jt, h, :],
                            expT[:, 512 * ih:512 * ih + 512],
                            start=(jt == 0), stop=(jt == NT - 1),
                        )
                if jt == 3 and filler is not None:
                    filler()
            for h in heads:
                q_off = (h % 2) * 64
                # free the PSUM slot fast: single copy of out^T + sums row
                o_tmp = p_otmp.tile([DH + 1, N], F32, tag="otmp",
                                    name=f"o_tmp_{b}_{h}")
                nc.vector.tensor_copy(o_tmp, ps_os[h])
                sums_sb = p_small.tile([1, N], F32, tag="sums")
                nc.vector.tensor_copy(sums_sb, o_tmp[DH:DH + 1, :])
                recip = p_small.tile([1, N], F32, tag="recip")
                nc.vector.reciprocal_approx_fast(recip, sums_sb)
                scr = p_dram.tile([1, N], F32, tag="scr")
                nc.gpsimd.dma_start(out=scr, in_=recip)
                rb = p_rb.tile([64, N], F32, tag="rb")
                nc.gpsimd.dma_start(
                    out=rb,
                    in_=bass.AP(tensor=scr.tensor, offset=scr.offset,
                                ap=[[0, 64], [1, N]]),
                )
                nc.vector.tensor_mul(
                    osb[b][q_off:q_off + 64, g, :], o_tmp[0:DH, :], rb
                )

        def emit_yproj_group(b, g):
            ps_y = psB.tile([128, N], F32, tag="psB")
            for rr in range(2):
                r = 2 * g + rr
                for kt in range(KD):
                    nc.tensor.matmul(
                        ps_y[:, 512 * rr:512 * rr + 512],
                        osb[b][:, kt, 128 * r:128 * r + 128],
                        wout_sb[:, kt, :],
                        start=(kt == 0), stop=(kt == KD - 1),
                    )
            for rr in range(2):
                r = 2 * g + rr
                y_sb = p_y.tile([128, D], F32, tag="y")
                nc.vector.tensor_add(
                    y_sb, ps_y[:, 512 * rr:512 * rr + 512], bout_bc
                )
                nc.sync.dma_start(
                    out=y[b, 128 * r:128 * r + 128, :], in_=y_sb
                )

        # ================= pipelined emission =================
        # batch 0 prologue: load + transpose + minimal projection prefix
        x0 = emit_load_x(0)
        xT[0] = p_mid.tile([128, KD, N], F32R, tag="mid", name="xT0")
        qkT[0] = p_qk.tile([128, 8, N], F32R, tag="qk", name="qkT0")
        vsb[0] = p_v.tile([128, NT, HEADS, DH + 1], F32R, tag="v", name="v0")
        osb[0] = p_mid.tile([128, KD, N], F32R, tag="mid", name="o0")
        emit_transposes(0, x0, range(KD))
        emit_ones(0)
        emit_qk_ct(0, 0)       # q heads 0,1
        emit_qk_ct(0, 4)       # k heads 0,1
        for g in range(4):
            emit_v_group(0, g)

        # batch 1 x-load can start as soon as x0's slot frees
        x1 = emit_load_x(1)
        xT[1] = p_mid.tile([128, KD, N], F32R, tag="mid", name="xT1")

        # C(0) pairs with remaining B(0) chunks + A(1) transposes interleaved
        fillers0 = [
            lambda: (emit_qk_ct(0, 1), emit_qk_ct(0, 5)),
            lambda: (emit_qk_ct(0, 2), emit_qk_ct(0, 6),
                     emit_transposes(1, x1, [0, 1])),
            lambda: (emit_qk_ct(0, 3), emit_qk_ct(0, 7),
                     emit_transposes(1, x1, [2, 3])),
            None,
        ]
        for g in range(4):
            emit_head_pair(0, g, filler=fillers0[g])

        # B(1) projections (PE-dense stretch between the two attention phases)
        qkT[1] = p_qk.tile([128, 8, N], F32R, tag="qk", name="qkT1")
        vsb[1] = p_v.tile([128, NT, HEADS, DH + 1], F32R, tag="v", name="v1")
        osb[1] = p_mid.tile([128, KD, N], F32R, tag="mid", name="o1")
        emit_ones(1)
        emit_qk_ct(1, 0)
        emit_qk_ct(1, 4)
        for g in range(4):
            emit_v_group(1, g)

        # C(1) pairs with D(0) + remaining B(1) chunks interleaved
        fillers1 = [
            lambda: (emit_qk_ct(1, 1), emit_qk_ct(1, 5),
                     emit_yproj_group(0, 0)),
            lambda: (emit_qk_ct(1, 2), emit_qk_ct(1, 6),
                     emit_yproj_group(0, 1)),
            lambda: (emit_qk_ct(1, 3), emit_qk_ct(1, 7),
                     emit_yproj_group(0, 2)),
            lambda: emit_yproj_group(0, 3),
        ]
        for g in range(4):
            emit_head_pair(1, g, filler=fillers1[g])

        # D(1) tail
        for g in range(4):
            emit_yproj_group(1, g)

    nc.compile()
    return nc


_NC = None


def _get_program():
    global _NC
    if _NC is None:
        _NC = build_program()
    return _NC


def make_in_maps(x, w_qkv, w_out, b_out, scale):
    x = np.ascontiguousarray(np.asarray(x, dtype=np.float32))
    w_qkv = np.ascontiguousarray(np.asarray(w_qkv, dtype=np.float32))
    w_out = np.ascontiguousarray(np.asarray(w_out, dtype=np.float32))
    b_out = np.ascontiguousarray(np.asarray(b_out, dtype=np.float32))
    scale = np.ascontiguousarray(np.asarray(scale, dtype=np.float32))
    return [
        {
            "x": x[c * BPC:(c + 1) * BPC],
            "w_qkv": w_qkv,
            "w_out": w_out,
            "b_out": b_out,
            "scale": scale,
        }
        for c in range(N_CORES)
    ]


def kernel(x, w_qkv, w_out, b_out, scale):
    nc = _get_program()
    in_maps = make_in_maps(x, w_qkv, w_out, b_out, scale)
    res = run_bass_kernel_spmd(nc, in_maps, core_ids=list(range(N_CORES)))
    return np.concatenate([res.results[c]["y"] for c in range(N_CORES)], axis=0)


if __name__ == "__main__":
    rng = np.random.default_rng(0)
    inputs = {
        "x": rng.standard_normal((B, N, D), dtype=np.float32),
        "w_qkv": rng.standard_normal((D, 3 * D), dtype=np.float32) * 0.03,
        "w_out": rng.standard_normal((D, D), dtype=np.float32) * 0.04,
        "b_out": np.zeros(D, dtype=np.float32),
        "scale": np.full(HEADS, DH ** -0.5, dtype=np.float32),
    }
    out = kernel(**inputs)
    print("kernel output", out.shape, out.dtype)
